# revision 26
# baseline (speedup 1.0000x reference)
"""CWICLinear TRN2 kernel.

Reference computation (per token row x of length I=4096, stripes n=0..7):
    xc   = x - mu
    mask = |xc| > thr_n          (thr_n = thresholds[n] * sqrt(I) * std)
    xm   = xc * mask + mu
    y[:, n*512:(n+1)*512] = xm @ W[:, n*512:(n+1)*512] + bias
    active = 512 * sum(mask over stripes+features)
    dense  = 512 * 8 * 4096 (constant)

Identity used on-chip:  y_n = (xc * mask_n) @ W_n + (mu @ W_n + bias_n)
The (mu @ W + bias) term and the (x - mu) transpose are host-side input prep.

Sharding (8 cores = 4 token-shards x 2 stripe-shards):
  each core: 512 tokens, 4 stripes (2048 out cols).

On-chip per core (f32r matmuls = TRN2 fast-fp32 mode, tf32-like rounding):
  xc loads feature-major [128, ko, tok]; a = |xc| via ACT (exact f32, so the
  mask compare is bit-identical to the reference).
  Per (stripe s, k-chunk k):
    m   = a_k > thr[s,k]      DVE tensor_scalar, bf16 0/1 out (exact)
    xcm = m * xc_k            DVE tensor_tensor -> f32r, batched per k-group
    psum[s,mo] += w[k,mo].T @ xcm    4 f32r matmuls (128x128 @ 128x512)
    pcnt[32kk] += ones_b.T @ m      bf16 count matmuls, col-tiled so the 4
                                    chunks of a group run concurrently on PE
  Epilogue per (s,mo): y = psum + cb (ACT bias-add) -> DMA out.
  Counts land in 4 PSUM rows (one per col-group), summed on host; exact.
"""

import numpy as np

import concourse.bass as bass  # noqa: F401
import concourse.tile as tile
from concourse import bacc, mybir
from concourse.bass_utils import run_bass_kernel_spmd

P = 128
IN_F = 4096
OUT_F = 4096
N_STR = 8
STR_SZ = OUT_F // N_STR  # 512
TOK = 2048
TW, SW = 4, 2            # token-ways x stripe-ways = 8 cores
T_C = TOK // TW          # 512 tokens per core
S_C = N_STR // SW        # 4 stripes per core
O_C = OUT_F // SW        # 2048 out cols per core
KO = IN_F // P           # 32 k-chunks
KPG = 4                  # k-chunks per group
KG = KO // KPG           # 8 groups
MO = STR_SZ // P         # 4 out-chunks per stripe
THR_SCALE = np.float32(IN_F**0.5)

_NC = None


def _build():
    f32, f32r, bf16 = mybir.dt.float32, mybir.dt.float32r, mybir.dt.bfloat16
    gt, mult = mybir.AluOpType.is_gt, mybir.AluOpType.mult
    AF = mybir.ActivationFunctionType
    nc = bacc.Bacc("TRN2", target_bir_lowering=False, debug=False, num_devices=8)
    xcT = nc.dram_tensor("xcT", [IN_F, T_C], f32, kind="ExternalInput")
    w = nc.dram_tensor("w", [IN_F, O_C], f32r, kind="ExternalInput")
    thr = nc.dram_tensor("thr", [P, S_C * KO], f32, kind="ExternalInput")
    cb = nc.dram_tensor("cb", [P, O_C // P], f32, kind="ExternalInput")
    yT = nc.dram_tensor("yT", [O_C, T_C], f32, kind="ExternalOutput")
    cnt = nc.dram_tensor("cnt", [1, KPG, T_C], f32, kind="ExternalOutput")

    with tile.TileContext(nc) as tc:
        with (
            tc.tile_pool(name="xcp", bufs=KG) as xc_pool,
            tc.tile_pool(name="ab", bufs=KG) as ab_pool,
            tc.tile_pool(name="wp", bufs=4) as w_pool,
            tc.tile_pool(name="mp", bufs=2) as m_pool,
            tc.tile_pool(name="xcmp", bufs=2) as xcm_pool,
            tc.tile_pool(name="yp", bufs=4) as y_pool,
            tc.tile_pool(name="consts", bufs=1) as c_pool,
            tc.tile_pool(name="psum", bufs=7, space="PSUM") as ps_pool,
            tc.tile_pool(name="pscnt", bufs=1, space="PSUM") as psc_pool,
        ):
            xc_re = xcT.ap().rearrange("(ko p) t -> p ko t", p=P)
            xc_tiles = [None] * KG
            a_tiles = [None] * KG

            # consts on the scalar (ACT) HWDGE ring so they don't queue behind W
            thr_sb = c_pool.tile([P, S_C * KO], f32, tag="thr")
            nc.scalar.dma_start(thr_sb[:], thr.ap())
            cb_sb = c_pool.tile([P, O_C // P], f32, tag="cb")
            nc.scalar.dma_start(cb_sb[:], cb.ap())
            ones_f = c_pool.tile([P, P], f32, tag="ones_f")
            nc.vector.memset(ones_f[:], 1.0)
            ones_r = c_pool.tile([P, P], f32r, tag="ones_r")
            nc.vector.tensor_scalar_mul(ones_r[:], ones_f[:], 1.0)
            ones_b = c_pool.tile([P, 1], bf16, tag="ones_b")
            nc.vector.tensor_copy(ones_b[:], ones_f[:, :1])

            def load_xc(g, split):
                # split=True streams the first group per-k so compute starts
                # after 256KB instead of 1MB; scalar ring keeps xc off W's FIFO
                t = xc_pool.tile([P, KPG, T_C], f32, tag="xc", name=f"xc{g}")
                a = ab_pool.tile([P, KPG, T_C], f32, tag="ab", name=f"ab{g}")
                src = xc_re[:, g * KPG : (g + 1) * KPG, :]
                for r in range(KPG) if split else [slice(None)]:
                    nc.gpsimd.dma_start(t[:, r], src[:, r])
                    nc.scalar.activation(a[:, r], t[:, r], AF.Abs)
                xc_tiles[g] = t
                a_tiles[g] = a

            w_re = w.ap().rearrange("(ko p) o -> p ko o", p=P)
            yT_re = yT.ap().rearrange("(j p) t -> j p t", p=P)
            pc = psc_pool.tile([P, T_C], f32)

            # group-0 activations + weights first: the first-matmul chain
            # (dma -> abs -> is_gt -> mult -> mm) starts during the preamble
            load_xc(0, split=True)
            wt0 = w_pool.tile([P, KPG, STR_SZ], f32r, tag="w", name="wt0")
            for kk in range(KPG):
                nc.sync.dma_start(wt0[:, kk], w_re[:, kk, 0:STR_SZ])

            # PE warm-up: dummy matmuls so HAM un-throttles the PE clock
            # before the real stream; overwritten by the first count matmuls.
            for r in range(48):
                nc.tensor.matmul(
                    pc[:1, :P], ones_r[:, :1], ones_r[:, :P], start=True, stop=True
                )

            for s in range(S_C):
                psums = [
                    ps_pool.tile([P, T_C], f32, tag="ps", name=f"ps_{s}_{mo}")
                    for mo in range(MO)
                ]
                for g in range(KG):
                    if s == 0 and g == 0:
                        wt = wt0
                    else:
                        wt = w_pool.tile([P, KPG, STR_SZ], f32r, tag="w")
                        nc.sync.dma_start(
                            wt[:],
                            w_re[:, g * KPG : (g + 1) * KPG, s * STR_SZ : (s + 1) * STR_SZ],
                        )
                    if s == 0 and g > 0:
                        load_xc(g, split=False)
                    mg = m_pool.tile([P, KPG, T_C], f32r, tag="m")
                    for kk in range(KPG):
                        k = g * KPG + kk
                        nc.vector.tensor_scalar(
                            mg[:, kk],
                            a_tiles[g][:, kk],
                            thr_sb[:, s * KO + k : s * KO + k + 1],
                            None,
                            gt,
                        )
                    xcm = xcm_pool.tile([P, KPG, T_C], f32r, tag="xcm")
                    if s == 0:
                        for kk in range(KPG):
                            nc.vector.tensor_tensor(
                                xcm[:, kk], mg[:, kk], xc_tiles[g][:, kk], mult
                            )
                    else:
                        nc.vector.tensor_tensor(
                            xcm[:], mg[:], xc_tiles[g][:], mult
                        )
                    # count matmuls first (need only the masks), batched per
                    # group: one bf16<->f32r weight mode switch per group and
                    # the 4 col-tiled MMs overlap on the PE array
                    for kk in range(KPG):
                        k = g * KPG + kk
                        nc.tensor.matmul(
                            pc[:1, :],
                            ones_r[:, :1],
                            mg[:, kk],
                            start=(s == 0 and k == 0),
                            stop=(s == S_C - 1 and k == KO - 1),
                        )
                    for kk in range(KPG):
                        k = g * KPG + kk
                        for mo in range(MO):
                            nc.tensor.matmul(
                                psums[mo][:],
                                wt[:, kk, mo * P : (mo + 1) * P],
                                xcm[:, kk],
                                start=(k == 0),
                                stop=(k == KO - 1),
                            )
                for mo in range(MO):
                    j = s * MO + mo
                    y = y_pool.tile([P, T_C], f32, tag="y")
                    nc.scalar.add(y[:], psums[mo][:], cb_sb[:, j : j + 1])
                    if s == S_C - 1:
                        nc.scalar.dma_start(yT_re[j], y[:])
                    else:
                        nc.gpsimd.dma_start(yT_re[j], y[:])

            # rows 1..3 of cnt stay zero (outputs are zero-initialized)
            cnt_sb = c_pool.tile([1, T_C], f32, tag="cnt")
            nc.vector.tensor_copy(cnt_sb[:], pc[:1, :])
            nc.gpsimd.dma_start(cnt.ap()[:, 0, :], cnt_sb[:])
    nc.compile()
    return nc


def _get_nc():
    global _NC
    if _NC is None:
        _NC = _build()
    return _NC


def make_in_maps(x, weight, bias, thresholds, mu, std):
    x32 = np.asarray(x, np.float32).reshape(TOK, IN_F)
    w32 = np.asarray(weight, np.float32)
    b32 = np.asarray(bias, np.float32)
    t32 = np.asarray(thresholds, np.float32)
    mu32 = np.asarray(mu, np.float32)
    sd32 = np.asarray(std, np.float32)

    xc = x32 - mu32[None, :]
    xcT = np.ascontiguousarray(xc.T)  # [IN_F, TOK]
    thrf = (t32 * THR_SCALE) * sd32[None, :]  # [8, IN_F], matches reference rounding
    c = (mu32 @ w32 + b32).astype(np.float32)  # [OUT_F]

    in_maps = []
    for core in range(8):
        si, ti = divmod(core, TW)
        xcT_c = np.ascontiguousarray(xcT[:, ti * T_C : (ti + 1) * T_C])
        w_c = np.ascontiguousarray(w32[:, si * O_C : (si + 1) * O_C])
        thr_rows = thrf[si * S_C : (si + 1) * S_C]  # [S_C, IN_F]
        thr_pack = np.ascontiguousarray(
            thr_rows.reshape(S_C, KO, P).transpose(2, 0, 1).reshape(P, S_C * KO)
        )
        c_c = c[si * O_C : (si + 1) * O_C]
        cb_pack = np.ascontiguousarray(c_c.reshape(O_C // P, P).T)
        in_maps.append({"xcT": xcT_c, "w": w_c, "thr": thr_pack, "cb": cb_pack})
    return in_maps


def assemble(results):
    y = np.empty((TOK, OUT_F), np.float32)
    cnt_acc = np.zeros((TW, T_C), np.float32)
    for core in range(8):
        si, ti = divmod(core, TW)
        y[ti * T_C : (ti + 1) * T_C, si * O_C : (si + 1) * O_C] = results[core]["yT"].T
        cnt_acc[ti] += results[core]["cnt"][0].sum(axis=0)
    active = (cnt_acc * np.float32(STR_SZ)).reshape(4, 512).astype(np.float32)
    dense = np.full((4, 512), np.float32(STR_SZ * N_STR * IN_F), np.float32)
    return y.reshape(4, 512, OUT_F), dense, active


def kernel(x, weight, bias, thresholds, mu, std):
    nc = _get_nc()
    in_maps = make_in_maps(x, weight, bias, thresholds, mu, std)
    res = run_bass_kernel_spmd(nc, in_maps, core_ids=list(range(8)))
    return assemble(res.results)


# revision 27
# speedup vs baseline: 1.0508x; 1.0508x over previous
"""CWICLinear TRN2 kernel.

Reference computation (per token row x of length I=4096, stripes n=0..7):
    xc   = x - mu
    mask = |xc| > thr_n          (thr_n = thresholds[n] * sqrt(I) * std)
    xm   = xc * mask + mu
    y[:, n*512:(n+1)*512] = xm @ W[:, n*512:(n+1)*512] + bias
    active = 512 * sum(mask over stripes+features)
    dense  = 512 * 8 * 4096 (constant)

Identity used on-chip:  y_n = (xc * mask_n) @ W_n + (mu @ W_n + bias_n)
The (mu @ W + bias) term and the (x - mu) transpose are host-side input prep.

Sharding (8 cores = 4 token-shards x 2 stripe-shards):
  each core: 512 tokens, 4 stripes (2048 out cols).

On-chip per core (f32r matmuls = TRN2 fast-fp32 mode, tf32-like rounding):
  xc loads feature-major [128, ko, tok]; a = |xc| via ACT (exact f32, so the
  mask compare is bit-identical to the reference).
  Per (stripe s, k-chunk k):
    m   = a_k > thr[s,k]      DVE tensor_scalar, bf16 0/1 out (exact)
    xcm = m * xc_k            DVE tensor_tensor -> f32r, batched per k-group
    psum[s,mo] += w[k,mo].T @ xcm    4 f32r matmuls (128x128 @ 128x512)
    pcnt[32kk] += ones_b.T @ m      bf16 count matmuls, col-tiled so the 4
                                    chunks of a group run concurrently on PE
  Epilogue per (s,mo): y = psum + cb (ACT bias-add) -> DMA out.
  Counts land in 4 PSUM rows (one per col-group), summed on host; exact.
"""

import numpy as np

import concourse.bass as bass  # noqa: F401
import concourse.tile as tile
from concourse import bacc, mybir
from concourse.bass_utils import run_bass_kernel_spmd

P = 128
IN_F = 4096
OUT_F = 4096
N_STR = 8
STR_SZ = OUT_F // N_STR  # 512
TOK = 2048
TW, SW = 4, 2            # token-ways x stripe-ways = 8 cores
T_C = TOK // TW          # 512 tokens per core
S_C = N_STR // SW        # 4 stripes per core
O_C = OUT_F // SW        # 2048 out cols per core
KO = IN_F // P           # 32 k-chunks
KPG = 4                  # k-chunks per group
KG = KO // KPG           # 8 groups
MO = STR_SZ // P         # 4 out-chunks per stripe
THR_SCALE = np.float32(IN_F**0.5)

_NC = None


def _build():
    f32, f32r, bf16 = mybir.dt.float32, mybir.dt.float32r, mybir.dt.bfloat16
    gt, mult = mybir.AluOpType.is_gt, mybir.AluOpType.mult
    AF = mybir.ActivationFunctionType
    nc = bacc.Bacc("TRN2", target_bir_lowering=False, debug=False, num_devices=8)
    xcT = nc.dram_tensor("xcT", [IN_F, T_C], f32, kind="ExternalInput")
    w = nc.dram_tensor("w", [IN_F, O_C], f32r, kind="ExternalInput")
    thr = nc.dram_tensor("thr", [P, S_C * KO], f32, kind="ExternalInput")
    cb = nc.dram_tensor("cb", [P, O_C // P], f32, kind="ExternalInput")
    yT = nc.dram_tensor("yT", [O_C, T_C], f32, kind="ExternalOutput")
    cnt = nc.dram_tensor("cnt", [1, KPG, T_C], f32, kind="ExternalOutput")

    with tile.TileContext(nc) as tc:
        with (
            tc.tile_pool(name="xcp", bufs=KG) as xc_pool,
            tc.tile_pool(name="ab", bufs=KG) as ab_pool,
            tc.tile_pool(name="wp", bufs=4) as w_pool,
            tc.tile_pool(name="mp", bufs=3) as m_pool,
            tc.tile_pool(name="xcmp", bufs=2) as xcm_pool,
            tc.tile_pool(name="yp", bufs=4) as y_pool,
            tc.tile_pool(name="consts", bufs=1) as c_pool,
            tc.tile_pool(name="psum", bufs=7, space="PSUM") as ps_pool,
            tc.tile_pool(name="pscnt", bufs=1, space="PSUM") as psc_pool,
        ):
            xc_re = xcT.ap().rearrange("(ko p) t -> p ko t", p=P)
            xc_tiles = [None] * KG
            a_tiles = [None] * KG

            # consts on the scalar (ACT) HWDGE ring so they don't queue behind W
            thr_sb = c_pool.tile([P, S_C * KO], f32, tag="thr")
            nc.scalar.dma_start(thr_sb[:], thr.ap())
            cb_sb = c_pool.tile([P, O_C // P], f32, tag="cb")
            nc.scalar.dma_start(cb_sb[:], cb.ap())
            ones_f = c_pool.tile([P, P], f32, tag="ones_f")
            nc.vector.memset(ones_f[:], 1.0)
            ones_r = c_pool.tile([P, P], f32r, tag="ones_r")
            nc.vector.tensor_scalar_mul(ones_r[:], ones_f[:], 1.0)
            ones_b = c_pool.tile([P, 1], bf16, tag="ones_b")
            nc.vector.tensor_copy(ones_b[:], ones_f[:, :1])

            def load_xc(g, split):
                # split=True streams the first group per-k so compute starts
                # after 256KB instead of 1MB; scalar ring keeps xc off W's FIFO
                t = xc_pool.tile([P, KPG, T_C], f32, tag="xc", name=f"xc{g}")
                a = ab_pool.tile([P, KPG, T_C], f32, tag="ab", name=f"ab{g}")
                src = xc_re[:, g * KPG : (g + 1) * KPG, :]
                for r in range(KPG) if split else [slice(None)]:
                    nc.gpsimd.dma_start(t[:, r], src[:, r])
                    nc.scalar.activation(a[:, r], t[:, r], AF.Abs)
                xc_tiles[g] = t
                a_tiles[g] = a

            w_re = w.ap().rearrange("(ko p) o -> p ko o", p=P)
            yT_re = yT.ap().rearrange("(j p) t -> j p t", p=P)
            pc = psc_pool.tile([P, T_C], f32)

            # group-0 activations + weights first: the first-matmul chain
            # (dma -> abs -> is_gt -> mult -> mm) starts during the preamble
            load_xc(0, split=True)
            wt0 = w_pool.tile([P, KPG, STR_SZ], f32r, tag="w", name="wt0")
            for kk in range(KPG):
                nc.sync.dma_start(wt0[:, kk], w_re[:, kk, 0:STR_SZ])

            # PE warm-up: dummy matmuls so HAM un-throttles the PE clock
            # before the real stream; overwritten by the first count matmuls.
            for r in range(48):
                nc.tensor.matmul(
                    pc[:1, :P], ones_r[:, :1], ones_r[:, :P], start=True, stop=True
                )

            for s in range(S_C):
                psums = [
                    ps_pool.tile([P, T_C], f32, tag="ps", name=f"ps_{s}_{mo}")
                    for mo in range(MO)
                ]
                for g in range(KG):
                    if s == 0 and g == 0:
                        wt = wt0
                    else:
                        wt = w_pool.tile([P, KPG, STR_SZ], f32r, tag="w")
                        nc.sync.dma_start(
                            wt[:],
                            w_re[:, g * KPG : (g + 1) * KPG, s * STR_SZ : (s + 1) * STR_SZ],
                        )
                    if s == 0 and g > 0:
                        load_xc(g, split=False)
                    mg = m_pool.tile([P, KPG, T_C], bf16, tag="m")
                    for kk in range(KPG):
                        k = g * KPG + kk
                        nc.vector.tensor_scalar(
                            mg[:, kk],
                            a_tiles[g][:, kk],
                            thr_sb[:, s * KO + k : s * KO + k + 1],
                            None,
                            gt,
                        )
                    xcm = xcm_pool.tile([P, KPG, T_C], f32r, tag="xcm")
                    if s == 0:
                        for kk in range(KPG):
                            nc.vector.tensor_tensor(
                                xcm[:, kk], mg[:, kk], xc_tiles[g][:, kk], mult
                            )
                    else:
                        nc.vector.tensor_tensor(
                            xcm[:], mg[:], xc_tiles[g][:], mult
                        )
                    # count matmuls first (need only the masks), batched per
                    # group: one bf16<->f32r weight mode switch per group and
                    # the 4 col-tiled MMs overlap on the PE array
                    for kk in range(KPG):
                        k = g * KPG + kk
                        nc.tensor.matmul(
                            pc[32 * kk : 32 * kk + 1, :],
                            ones_b[:],
                            mg[:, kk],
                            start=(s == 0 and k < KPG),
                            stop=(s == S_C - 1 and k >= KO - KPG),
                            tile_position=(0, 32 * kk),
                        )
                    for kk in range(KPG):
                        k = g * KPG + kk
                        for mo in range(MO):
                            nc.tensor.matmul(
                                psums[mo][:],
                                wt[:, kk, mo * P : (mo + 1) * P],
                                xcm[:, kk],
                                start=(k == 0),
                                stop=(k == KO - 1),
                            )
                for mo in range(MO):
                    j = s * MO + mo
                    y = y_pool.tile([P, T_C], f32, tag="y")
                    nc.scalar.add(y[:], psums[mo][:], cb_sb[:, j : j + 1])
                    if s == S_C - 1:
                        nc.scalar.dma_start(yT_re[j], y[:])
                    else:
                        nc.gpsimd.dma_start(yT_re[j], y[:])

            cnt_sb = c_pool.tile([1, KPG, T_C], f32, tag="cnt")
            for kk in range(KPG):
                nc.vector.tensor_copy(cnt_sb[:, kk, :], pc[32 * kk : 32 * kk + 1, :])
            nc.gpsimd.dma_start(cnt.ap(), cnt_sb[:])
    nc.compile()
    return nc


def _get_nc():
    global _NC
    if _NC is None:
        _NC = _build()
    return _NC


def make_in_maps(x, weight, bias, thresholds, mu, std):
    x32 = np.asarray(x, np.float32).reshape(TOK, IN_F)
    w32 = np.asarray(weight, np.float32)
    b32 = np.asarray(bias, np.float32)
    t32 = np.asarray(thresholds, np.float32)
    mu32 = np.asarray(mu, np.float32)
    sd32 = np.asarray(std, np.float32)

    xc = x32 - mu32[None, :]
    xcT = np.ascontiguousarray(xc.T)  # [IN_F, TOK]
    thrf = (t32 * THR_SCALE) * sd32[None, :]  # [8, IN_F], matches reference rounding
    c = (mu32 @ w32 + b32).astype(np.float32)  # [OUT_F]

    in_maps = []
    for core in range(8):
        si, ti = divmod(core, TW)
        xcT_c = np.ascontiguousarray(xcT[:, ti * T_C : (ti + 1) * T_C])
        w_c = np.ascontiguousarray(w32[:, si * O_C : (si + 1) * O_C])
        thr_rows = thrf[si * S_C : (si + 1) * S_C]  # [S_C, IN_F]
        thr_pack = np.ascontiguousarray(
            thr_rows.reshape(S_C, KO, P).transpose(2, 0, 1).reshape(P, S_C * KO)
        )
        c_c = c[si * O_C : (si + 1) * O_C]
        cb_pack = np.ascontiguousarray(c_c.reshape(O_C // P, P).T)
        in_maps.append({"xcT": xcT_c, "w": w_c, "thr": thr_pack, "cb": cb_pack})
    return in_maps


def assemble(results):
    y = np.empty((TOK, OUT_F), np.float32)
    cnt_acc = np.zeros((TW, T_C), np.float32)
    for core in range(8):
        si, ti = divmod(core, TW)
        y[ti * T_C : (ti + 1) * T_C, si * O_C : (si + 1) * O_C] = results[core]["yT"].T
        cnt_acc[ti] += results[core]["cnt"][0].sum(axis=0)
    active = (cnt_acc * np.float32(STR_SZ)).reshape(4, 512).astype(np.float32)
    dense = np.full((4, 512), np.float32(STR_SZ * N_STR * IN_F), np.float32)
    return y.reshape(4, 512, OUT_F), dense, active


def kernel(x, weight, bias, thresholds, mu, std):
    nc = _get_nc()
    in_maps = make_in_maps(x, weight, bias, thresholds, mu, std)
    res = run_bass_kernel_spmd(nc, in_maps, core_ids=list(range(8)))
    return assemble(res.results)
